# revision 3
# baseline (speedup 1.0000x reference)
"""Per-pixel adaptive 5x5 conv (KPN) for Trainium2, 8-core data parallel.

out[g,h,w] = sum_{i,j} core[g,5i+j,h,w] * frames_pad[g,h+i-2,w+j-2]
with g = flattened (B,N) = 16 image planes; 2 planes per NeuronCore.

Engine split (v3):
  DVE    : only the 25 per-tap multiplies per image (fp16 2x_1P mode).
  TensorE: all tap accumulation, via identity-matmul into PSUM (fp32
           accumulate; 4 matmuls of FD=512 per tap = 1 PSUM bank each).
  ScalarE: per-bank PSUM fp32 -> SBUF fp16 casts + output DMA ring
           (ACT HWDGE) so stores don't queue behind the weight stream.
  GpSimd : idle (shares SBUF ports with DVE; using it slows DVE).

The kernel is paced by the single-queue HBM stream (~409 GB/s): 30.5 MB
per core. DMA chunking is sized so compute tracks the stream with ~1 us
lag: frames go in two par-halves, the first weight group of img0 and the
last of img1 go per-tap (0.52 MB), the bulk in 5-tap groups (2.62 MB).

Layout: rows interleaved 4-per-partition. Partition p holds padded rows
4p..4p+7 (= orig rows 4p-2..4p+5), so ALL row shifts i=0..4 are free-dim
offsets -- no cross-partition moves and no per-shift duplication.
  fin [2, 2, 128, 8*518]: fin[img,par,p,row*518+col] = Fpad[img,4p+row,col+(1-par)]
     two column-parity copies keep every tap's 512-col slice 4-byte
     aligned so DVE 2x fp16 mode engages for all 25 taps.
  win [2, 5, 128, 5*4*512]: win[img,tg,p,k,r,c] = core[img,5tg+k,4p+r,c]
  oout [2, 128, 4*512] fp16: oout[img,p,r*512+c] = out[img,4p+r,c]
"""

import os
import sys

import numpy as np

for _p in ("/opt/trn_rl_repo",):
    if _p not in sys.path and os.path.isdir(_p):
        sys.path.insert(0, _p)

K = 5
NCORES = 8
IMGS_PER_CORE = 2
H = W = 512
RPP = 4          # output rows per partition
FROWS = RPP + K - 1  # 8 padded rows held per partition
FCOLS = 518
FH_FREE = FROWS * FCOLS  # 4144 elems per parity half
W_FREE = K * RPP * W  # 10240
T_FREE = RPP * W  # 2048 (one tap's weights / one product / output)

_compiled = {}
last_results = None  # BassKernelResults of the most recent run (for test.py)


def _build_nc():
    import concourse.bacc as bacc
    import concourse.mybir as mybir
    from concourse.tile import TileContext

    f16 = mybir.dt.float16
    f32 = mybir.dt.float32

    nc = bacc.Bacc(None, target_bir_lowering=False, debug=False)
    fin = nc.dram_tensor("fin", [IMGS_PER_CORE, 2, 128, FH_FREE], f16,
                         kind="ExternalInput")
    win = nc.dram_tensor("win", [IMGS_PER_CORE, K, 128, W_FREE], f16,
                         kind="ExternalInput")
    iden = nc.dram_tensor("iden", [128, 128], f16, kind="ExternalInput")
    oout = nc.dram_tensor("oout", [IMGS_PER_CORE, 128, T_FREE], f16,
                          kind="ExternalOutput")

    with TileContext(nc) as tc:
        with (
            tc.tile_pool(name="idp", bufs=1) as idp,
            tc.tile_pool(name="fpool", bufs=1) as fpool,
            tc.tile_pool(name="wgrp", bufs=3) as wgrp,
            tc.tile_pool(name="wtap", bufs=6) as wtap,
            tc.tile_pool(name="prpool", bufs=4) as prpool,
            tc.tile_pool(name="opool", bufs=1) as opool,
            tc.psum_pool(name="ppool", bufs=1) as ppool,
        ):
            id_t = idp.tile([128, 128], f16)
            nc.scalar.dma_start(out=id_t[:], in_=iden[:])

            for img in range(IMGS_PER_CORE):
                # which weight groups stream per-tap (ramp-in / drain-out)
                split_tg = {0} if img == 0 else {K - 1}

                f_t = fpool.tile([128, 2 * FH_FREE], f16, tag=f"fin{img}")
                fv = f_t[:].rearrange("p (par row col) -> p par row col",
                                      par=2, row=FROWS, col=FCOLS)
                if img == 0:
                    # par0 first; tap0 (j=0) needs only par0. par1 is
                    # interleaved after the first weight tap below.
                    nc.sync.dma_start(out=f_t[:, 0:FH_FREE], in_=fin[img, 0])
                else:
                    nc.sync.dma_start(out=f_t[:, 0:FH_FREE], in_=fin[img, 0])
                    nc.sync.dma_start(out=f_t[:, FH_FREE:], in_=fin[img, 1])

                ps = ppool.tile([128, T_FREE], f32, tag=f"ps{img}")

                for tg in range(K):
                    if tg in split_tg:
                        w_ts = []
                        for k in range(K):
                            w_t = wtap.tile([128, T_FREE], f16, tag="wt")
                            nc.sync.dma_start(
                                out=w_t[:],
                                in_=win[img, tg][:, k * T_FREE:(k + 1) * T_FREE])
                            w_ts.append(w_t)
                            if img == 0 and tg == 0 and k == 0:
                                nc.sync.dma_start(out=f_t[:, FH_FREE:],
                                                  in_=fin[img, 1])
                        wviews = [w_ts[k][:].rearrange(
                            "p (r c) -> p r c", r=RPP) for k in range(K)]
                    else:
                        w_t = wgrp.tile([128, W_FREE], f16, tag="wg")
                        nc.sync.dma_start(out=w_t[:], in_=win[img, tg])
                        wv = w_t[:].rearrange("p (k r c) -> p k r c",
                                              k=K, r=RPP, c=W)
                        wviews = [wv[:, k] for k in range(K)]

                    for k in range(K):
                        t = tg * K + k
                        i, j = divmod(t, K)
                        par = j & 1
                        joff = j + par
                        prod = prpool.tile([128, T_FREE], f16, tag="pr")
                        pv = prod[:].rearrange("p (r c) -> p r c", r=RPP)
                        f_ap = fv[:, par, i:i + RPP, joff:joff + W]
                        nc.vector.tensor_mul(out=pv, in0=wviews[k], in1=f_ap)
                        for b in range(RPP):
                            nc.tensor.matmul(
                                ps[:, b * W:(b + 1) * W],
                                id_t[:],
                                prod[:, b * W:(b + 1) * W],
                                start=(t == 0),
                                stop=(t == K * K - 1),
                            )

                o_t = opool.tile([128, T_FREE], f16, tag=f"o{img}")
                for b in range(RPP):
                    nc.scalar.copy(out=o_t[:, b * W:(b + 1) * W],
                                   in_=ps[:, b * W:(b + 1) * W])
                nc.scalar.dma_start(out=oout[img], in_=o_t[:])
    nc.finalize()
    return nc


def _host_prep(frames, core):
    """Build per-core in_maps. frames [4,4,1,512,512] f32, core [4,4,25,1,512,512]."""
    G = NCORES * IMGS_PER_CORE  # 16
    F = np.ascontiguousarray(frames.reshape(G, H, W))
    Wc = core.reshape(G, K * K, H, W)

    # frames: Fpad[g, R, C] = F[g, R-2, C-3]; rows pad 2/2, cols 3/4
    Fp = np.pad(F, ((0, 0), (2, 2), (3, 4))).astype(np.float16)  # [G,516,519]
    # 8-row windows starting at every 4th row: sw[g, p, row, col] = Fp[g, 4p+row, col]
    sw = np.lib.stride_tricks.sliding_window_view(Fp, FROWS, axis=1)
    sw = sw[:, ::RPP].transpose(0, 1, 3, 2)  # [G, 128, 8, 519]
    fprep = np.empty((G, 2, 128, FROWS, FCOLS), np.float16)
    fprep[:, 0] = sw[..., 1:1 + FCOLS]   # par=0: Fpad col c+1
    fprep[:, 1] = sw[..., 0:FCOLS]       # par=1: Fpad col c

    # weights: win[g, tg, p, k, r, c] = core[g, 5tg+k, 4p+r, c]
    w16 = Wc.astype(np.float16)
    wprep = w16.reshape(G, K, K, 128, RPP, W).transpose(0, 1, 3, 2, 4, 5)

    iden = np.eye(128, dtype=np.float16)
    in_maps = []
    for c in range(NCORES):
        g0 = c * IMGS_PER_CORE
        in_maps.append({
            "fin": np.ascontiguousarray(
                fprep[g0:g0 + IMGS_PER_CORE].reshape(IMGS_PER_CORE, 2, 128, FH_FREE)),
            "win": np.ascontiguousarray(
                wprep[g0:g0 + IMGS_PER_CORE].reshape(IMGS_PER_CORE, K, 128, W_FREE)),
            "iden": iden,
        })
    return in_maps


def kernel(frames, core, bias):
    global last_results
    from concourse.bass_utils import run_bass_kernel_spmd

    frames = np.asarray(frames, dtype=np.float32)
    core = np.asarray(core, dtype=np.float32)

    if "nc" not in _compiled:
        _compiled["nc"] = _build_nc()
    nc = _compiled["nc"]

    in_maps = _host_prep(frames, core)
    trace = os.environ.get("KC_TRACE") == "1"
    tmpdir = os.environ.get("KC_TRACE_DIR") or None
    if tmpdir:
        os.makedirs(tmpdir, exist_ok=True)
    res = run_bass_kernel_spmd(nc, in_maps, list(range(NCORES)), trace=trace,
                               tmpdir=tmpdir)
    last_results = res

    G = NCORES * IMGS_PER_CORE
    out = np.empty((G, H, W), np.float32)
    for c in range(NCORES):
        o = res.results[c]["oout"]  # [2, 128, 2048] f16; rows are 4p+r in order
        for img in range(IMGS_PER_CORE):
            out[c * IMGS_PER_CORE + img] = o[img].reshape(H, W).astype(np.float32)
    return out.reshape(4, 4, H, W)


# revision 4
# speedup vs baseline: 1.0023x; 1.0023x over previous
"""Per-pixel adaptive 5x5 conv (KPN) for Trainium2, 8-core data parallel.

out[g,h,w] = sum_{i,j} core[g,5i+j,h,w] * frames_pad[g,h+i-2,w+j-2]
with g = flattened (B,N) = 16 image planes; 2 planes per NeuronCore.

Engine split (v3):
  DVE    : only the 25 per-tap multiplies per image (fp16 2x_1P mode).
  TensorE: all tap accumulation, via identity-matmul into PSUM (fp32
           accumulate; 4 matmuls of FD=512 per tap = 1 PSUM bank each).
  ScalarE: per-bank PSUM fp32 -> SBUF fp16 casts + output DMA ring
           (ACT HWDGE) so stores don't queue behind the weight stream.
  GpSimd : idle (shares SBUF ports with DVE; using it slows DVE).

The kernel is paced by the single-queue HBM stream (~409 GB/s): 30.5 MB
per core. DMA chunking is sized so compute tracks the stream with ~1 us
lag: frames go in two par-halves, the first weight group of img0 and the
last of img1 go per-tap (0.52 MB), the bulk in 5-tap groups (2.62 MB).

Layout: rows interleaved 4-per-partition. Partition p holds padded rows
4p..4p+7 (= orig rows 4p-2..4p+5), so ALL row shifts i=0..4 are free-dim
offsets -- no cross-partition moves and no per-shift duplication.
  fin [2, 2, 128, 8*518]: fin[img,par,p,row*518+col] = Fpad[img,4p+row,col+(1-par)]
     two column-parity copies keep every tap's 512-col slice 4-byte
     aligned so DVE 2x fp16 mode engages for all 25 taps.
  win [2, 5, 128, 5*4*512]: win[img,tg,p,k,r,c] = core[img,5tg+k,4p+r,c]
  oout [2, 128, 4*512] fp16: oout[img,p,r*512+c] = out[img,4p+r,c]
"""

import os
import sys

import numpy as np

for _p in ("/opt/trn_rl_repo",):
    if _p not in sys.path and os.path.isdir(_p):
        sys.path.insert(0, _p)

K = 5
NCORES = 8
IMGS_PER_CORE = 2
H = W = 512
RPP = 4          # output rows per partition
FROWS = RPP + K - 1  # 8 padded rows held per partition
FCOLS = 518
FH_FREE = FROWS * FCOLS  # 4144 elems per parity half
W_FREE = K * RPP * W  # 10240
T_FREE = RPP * W  # 2048 (one tap's weights / one product / output)

_compiled = {}
last_results = None  # BassKernelResults of the most recent run (for test.py)


def _build_nc():
    import concourse.bacc as bacc
    import concourse.mybir as mybir
    from concourse.tile import TileContext

    f16 = mybir.dt.float16
    f32 = mybir.dt.float32

    nc = bacc.Bacc(None, target_bir_lowering=False, debug=False)
    fin = nc.dram_tensor("fin", [IMGS_PER_CORE, 2, 128, FH_FREE], f16,
                         kind="ExternalInput")
    win = nc.dram_tensor("win", [IMGS_PER_CORE, K, 128, W_FREE], f16,
                         kind="ExternalInput")
    iden = nc.dram_tensor("iden", [128, 128], f16, kind="ExternalInput")
    oout = nc.dram_tensor("oout", [IMGS_PER_CORE, 128, T_FREE], f16,
                          kind="ExternalOutput")

    with TileContext(nc) as tc:
        with (
            tc.tile_pool(name="idp", bufs=1) as idp,
            tc.tile_pool(name="fpool", bufs=1) as fpool,
            tc.tile_pool(name="wgrp", bufs=5) as wgrp,
            tc.tile_pool(name="wtap", bufs=6) as wtap,
            tc.tile_pool(name="prpool", bufs=6) as prpool,
            tc.tile_pool(name="opool", bufs=1) as opool,
            tc.psum_pool(name="ppool", bufs=1) as ppool,
        ):
            id_t = idp.tile([128, 128], f16)
            nc.scalar.dma_start(out=id_t[:], in_=iden[:])

            for img in range(IMGS_PER_CORE):
                # which weight groups stream per-tap (ramp-in / drain-out)
                split_tg = {0} if img == 0 else {K - 1}

                f_t = fpool.tile([128, 2 * FH_FREE], f16, tag=f"fin{img}")
                fv = f_t[:].rearrange("p (par row col) -> p par row col",
                                      par=2, row=FROWS, col=FCOLS)
                if img == 0:
                    # par0 first; tap0 (j=0) needs only par0. par1 is
                    # interleaved after the first weight tap below.
                    nc.sync.dma_start(out=f_t[:, 0:FH_FREE], in_=fin[img, 0])
                else:
                    nc.sync.dma_start(out=f_t[:, 0:FH_FREE], in_=fin[img, 0])
                    nc.sync.dma_start(out=f_t[:, FH_FREE:], in_=fin[img, 1])

                ps = ppool.tile([128, T_FREE], f32, tag=f"ps{img}")

                for tg in range(K):
                    if tg in split_tg:
                        w_ts = []
                        for k in range(K):
                            w_t = wtap.tile([128, T_FREE], f16, tag="wt")
                            nc.sync.dma_start(
                                out=w_t[:],
                                in_=win[img, tg][:, k * T_FREE:(k + 1) * T_FREE])
                            w_ts.append(w_t)
                            if img == 0 and tg == 0 and k == 0:
                                nc.sync.dma_start(out=f_t[:, FH_FREE:],
                                                  in_=fin[img, 1])
                        wviews = [w_ts[k][:].rearrange(
                            "p (r c) -> p r c", r=RPP) for k in range(K)]
                    else:
                        w_t = wgrp.tile([128, W_FREE], f16, tag="wg")
                        nc.sync.dma_start(out=w_t[:], in_=win[img, tg])
                        wv = w_t[:].rearrange("p (k r c) -> p k r c",
                                              k=K, r=RPP, c=W)
                        wviews = [wv[:, k] for k in range(K)]

                    for k in range(K):
                        t = tg * K + k
                        i, j = divmod(t, K)
                        par = j & 1
                        joff = j + par
                        prod = prpool.tile([128, T_FREE], f16, tag="pr")
                        pv = prod[:].rearrange("p (r c) -> p r c", r=RPP)
                        f_ap = fv[:, par, i:i + RPP, joff:joff + W]
                        nc.vector.tensor_mul(out=pv, in0=wviews[k], in1=f_ap)
                        for b in range(RPP):
                            nc.tensor.matmul(
                                ps[:, b * W:(b + 1) * W],
                                id_t[:],
                                prod[:, b * W:(b + 1) * W],
                                start=(t == 0),
                                stop=(t == K * K - 1),
                            )

                o_t = opool.tile([128, T_FREE], f16, tag=f"o{img}")
                for b in range(RPP):
                    nc.scalar.copy(out=o_t[:, b * W:(b + 1) * W],
                                   in_=ps[:, b * W:(b + 1) * W])
                nc.scalar.dma_start(out=oout[img], in_=o_t[:])
    nc.finalize()
    return nc


def _host_prep(frames, core):
    """Build per-core in_maps. frames [4,4,1,512,512] f32, core [4,4,25,1,512,512]."""
    G = NCORES * IMGS_PER_CORE  # 16
    F = np.ascontiguousarray(frames.reshape(G, H, W))
    Wc = core.reshape(G, K * K, H, W)

    # frames: Fpad[g, R, C] = F[g, R-2, C-3]; rows pad 2/2, cols 3/4
    Fp = np.pad(F, ((0, 0), (2, 2), (3, 4))).astype(np.float16)  # [G,516,519]
    # 8-row windows starting at every 4th row: sw[g, p, row, col] = Fp[g, 4p+row, col]
    sw = np.lib.stride_tricks.sliding_window_view(Fp, FROWS, axis=1)
    sw = sw[:, ::RPP].transpose(0, 1, 3, 2)  # [G, 128, 8, 519]
    fprep = np.empty((G, 2, 128, FROWS, FCOLS), np.float16)
    fprep[:, 0] = sw[..., 1:1 + FCOLS]   # par=0: Fpad col c+1
    fprep[:, 1] = sw[..., 0:FCOLS]       # par=1: Fpad col c

    # weights: win[g, tg, p, k, r, c] = core[g, 5tg+k, 4p+r, c]
    w16 = Wc.astype(np.float16)
    wprep = w16.reshape(G, K, K, 128, RPP, W).transpose(0, 1, 3, 2, 4, 5)

    iden = np.eye(128, dtype=np.float16)
    in_maps = []
    for c in range(NCORES):
        g0 = c * IMGS_PER_CORE
        in_maps.append({
            "fin": np.ascontiguousarray(
                fprep[g0:g0 + IMGS_PER_CORE].reshape(IMGS_PER_CORE, 2, 128, FH_FREE)),
            "win": np.ascontiguousarray(
                wprep[g0:g0 + IMGS_PER_CORE].reshape(IMGS_PER_CORE, K, 128, W_FREE)),
            "iden": iden,
        })
    return in_maps


def kernel(frames, core, bias):
    global last_results
    from concourse.bass_utils import run_bass_kernel_spmd

    frames = np.asarray(frames, dtype=np.float32)
    core = np.asarray(core, dtype=np.float32)

    if "nc" not in _compiled:
        _compiled["nc"] = _build_nc()
    nc = _compiled["nc"]

    in_maps = _host_prep(frames, core)
    trace = os.environ.get("KC_TRACE") == "1"
    tmpdir = os.environ.get("KC_TRACE_DIR") or None
    if tmpdir:
        os.makedirs(tmpdir, exist_ok=True)
    res = run_bass_kernel_spmd(nc, in_maps, list(range(NCORES)), trace=trace,
                               tmpdir=tmpdir)
    last_results = res

    G = NCORES * IMGS_PER_CORE
    out = np.empty((G, H, W), np.float32)
    for c in range(NCORES):
        o = res.results[c]["oout"]  # [2, 128, 2048] f16; rows are 4p+r in order
        for img in range(IMGS_PER_CORE):
            out[c * IMGS_PER_CORE + img] = o[img].reshape(H, W).astype(np.float32)
    return out.reshape(4, 4, H, W)


# revision 5
# speedup vs baseline: 1.0646x; 1.0621x over previous
"""Per-pixel adaptive 5x5 conv (KPN) for Trainium2, 8-core data parallel.

out[g,h,w] = sum_{i,j} core[g,5i+j,h,w] * frames_pad[g,h+i-2,w+j-2]
with g = flattened (B,N) = 16 image planes; 2 planes per NeuronCore.

Engine split (v5):
  DVE    : only the 25 per-tap multiplies per image (fp16 2x_1P mode).
  TensorE: all tap accumulation, via identity-matmul into PSUM (fp32
           accumulate; 4 matmuls of FD=512 per tap = 1 PSUM bank each).
  ScalarE: builds the odd-parity frame copy on-chip (saves 2.1 MB of HBM
           per core), per-bank PSUM fp32 -> fp16 casts, output DMA ring.
  GpSimd : idle (shares SBUF ports with DVE; using it slows DVE).

The end-to-end time is paced by the weight stream through the 16 SDMA
engines (one of which runs ~17% slower and gates every completion sem),
so the kernel minimizes streamed bytes and DMA instruction count, and
chunks only the last weight group per-tap so the post-stream tail is one
tap, not five. Taps go even-j first so the on-chip parity copy has slack.

Layout: rows interleaved 4-per-partition. Partition p holds padded rows
4p..4p+7 (= orig rows 4p-2..4p+5), so ALL row shifts i=0..4 are free-dim
offsets -- no cross-partition moves and no per-shift duplication.
  fin [2, 128, 8*518]: fin[img,p,row*518+col] = Fpad[img,4p+row,col+1]
     (par0). The par1 copy (shift by one col) is made on-chip; its col 0
     is never read so the copy writes cols 1..4143 only.
  win [2, 5, 128, 5*4*512]: win[img,tg,p,k,r,c] = core[img,5tg+k,4p+r,c]
  oout [2, 128, 4*512] fp16: oout[img,p,r*512+c] = out[img,4p+r,c]
"""

import os
import sys

import numpy as np

for _p in ("/opt/trn_rl_repo",):
    if _p not in sys.path and os.path.isdir(_p):
        sys.path.insert(0, _p)

K = 5
NCORES = 8
IMGS_PER_CORE = 2
H = W = 512
RPP = 4          # output rows per partition
FROWS = RPP + K - 1  # 8 padded rows held per partition
FCOLS = 518
FH_FREE = FROWS * FCOLS  # 4144 elems per parity copy
W_FREE = K * RPP * W  # 10240
T_FREE = RPP * W  # 2048 (one tap's weights / one product / output)
KORDER = [0, 2, 4, 1, 3]  # even-j taps first within each group

_compiled = {}
last_results = None  # BassKernelResults of the most recent run (for test.py)


def _build_nc():
    import concourse.bacc as bacc
    import concourse.mybir as mybir
    from concourse.tile import TileContext

    f16 = mybir.dt.float16
    f32 = mybir.dt.float32

    nc = bacc.Bacc(None, target_bir_lowering=False, debug=False)
    fin = nc.dram_tensor("fin", [IMGS_PER_CORE, 128, FH_FREE], f16,
                         kind="ExternalInput")
    win = nc.dram_tensor("win", [IMGS_PER_CORE, K, 128, W_FREE], f16,
                         kind="ExternalInput")
    iden = nc.dram_tensor("iden", [128, 128], f16, kind="ExternalInput")
    oout = nc.dram_tensor("oout", [IMGS_PER_CORE, 128, T_FREE], f16,
                          kind="ExternalOutput")

    with TileContext(nc) as tc:
        with (
            tc.tile_pool(name="idp", bufs=1) as idp,
            tc.tile_pool(name="fpool", bufs=1) as fpool,
            tc.tile_pool(name="wgrp", bufs=5) as wgrp,
            tc.tile_pool(name="wtap", bufs=5) as wtap,
            tc.tile_pool(name="prpool", bufs=6) as prpool,
            tc.tile_pool(name="opool", bufs=1) as opool,
            tc.psum_pool(name="ppool", bufs=1) as ppool,
        ):
            id_t = idp.tile([128, 128], f16)
            nc.scalar.dma_start(out=id_t[:], in_=iden[:])

            n_emitted = 0
            for img in range(IMGS_PER_CORE):
                split_tg = {K - 1} if img == IMGS_PER_CORE - 1 else set()

                f0_t = fpool.tile([128, FH_FREE], f16, tag=f"f0_{img}")
                nc.sync.dma_start(out=f0_t[:], in_=fin[img])
                # on-chip odd-parity copy: par1[c] = par0[c-1]; col 0 unread
                f1_t = fpool.tile([128, FH_FREE], f16, tag=f"f1_{img}")
                nc.scalar.copy(out=f1_t[:, 1:FH_FREE],
                               in_=f0_t[:, 0:FH_FREE - 1])
                fviews = [
                    f0_t[:].rearrange("p (row col) -> p row col", col=FCOLS),
                    f1_t[:].rearrange("p (row col) -> p row col", col=FCOLS),
                ]

                ps = ppool.tile([128, T_FREE], f32, tag=f"ps{img}")

                for tg in range(K):
                    if tg in split_tg:
                        w_ts = {}
                        for k in KORDER:
                            w_t = wtap.tile([128, T_FREE], f16, tag="wt")
                            nc.sync.dma_start(
                                out=w_t[:],
                                in_=win[img, tg][:, k * T_FREE:(k + 1) * T_FREE])
                            w_ts[k] = w_t
                        wviews = {k: w_ts[k][:].rearrange(
                            "p (r c) -> p r c", r=RPP) for k in KORDER}
                    else:
                        w_t = wgrp.tile([128, W_FREE], f16, tag="wg")
                        nc.sync.dma_start(out=w_t[:], in_=win[img, tg])
                        wv = w_t[:].rearrange("p (k r c) -> p k r c",
                                              k=K, r=RPP, c=W)
                        wviews = {k: wv[:, k] for k in KORDER}

                    for k in KORDER:
                        i, j = tg, k
                        par = j & 1
                        joff = j + par
                        prod = prpool.tile([128, T_FREE], f16, tag="pr")
                        pv = prod[:].rearrange("p (r c) -> p r c", r=RPP)
                        f_ap = fviews[par][:, i:i + RPP, joff:joff + W]
                        nc.vector.tensor_mul(out=pv, in0=wviews[k], in1=f_ap)
                        for b in range(RPP):
                            nc.tensor.matmul(
                                ps[:, b * W:(b + 1) * W],
                                id_t[:],
                                prod[:, b * W:(b + 1) * W],
                                start=(n_emitted == 0),
                                stop=(n_emitted == K * K - 1),
                            )
                        n_emitted += 1
                n_emitted = 0

                o_t = opool.tile([128, T_FREE], f16, tag=f"o{img}")
                for b in range(RPP):
                    nc.scalar.copy(out=o_t[:, b * W:(b + 1) * W],
                                   in_=ps[:, b * W:(b + 1) * W])
                nc.scalar.dma_start(out=oout[img], in_=o_t[:])
    nc.finalize()
    return nc


def _host_prep(frames, core):
    """Build per-core in_maps. frames [4,4,1,512,512] f32, core [4,4,25,1,512,512]."""
    G = NCORES * IMGS_PER_CORE  # 16
    F = np.ascontiguousarray(frames.reshape(G, H, W))
    Wc = core.reshape(G, K * K, H, W)

    # frames: Fpad[g, R, C] = F[g, R-2, C-3]; rows pad 2/2, cols 3/4
    Fp = np.pad(F, ((0, 0), (2, 2), (3, 4))).astype(np.float16)  # [G,516,519]
    # 8-row windows starting at every 4th row: sw[g, p, row, col] = Fp[g, 4p+row, col]
    sw = np.lib.stride_tricks.sliding_window_view(Fp, FROWS, axis=1)
    sw = sw[:, ::RPP].transpose(0, 1, 3, 2)  # [G, 128, 8, 519]
    fprep = np.ascontiguousarray(sw[..., 1:1 + FCOLS])  # par=0: Fpad col c+1

    # weights: win[g, tg, p, k, r, c] = core[g, 5tg+k, 4p+r, c]
    w16 = Wc.astype(np.float16)
    wprep = w16.reshape(G, K, K, 128, RPP, W).transpose(0, 1, 3, 2, 4, 5)

    iden = np.eye(128, dtype=np.float16)
    in_maps = []
    for c in range(NCORES):
        g0 = c * IMGS_PER_CORE
        in_maps.append({
            "fin": np.ascontiguousarray(
                fprep[g0:g0 + IMGS_PER_CORE].reshape(IMGS_PER_CORE, 128, FH_FREE)),
            "win": np.ascontiguousarray(
                wprep[g0:g0 + IMGS_PER_CORE].reshape(IMGS_PER_CORE, K, 128, W_FREE)),
            "iden": iden,
        })
    return in_maps


def kernel(frames, core, bias):
    global last_results
    from concourse.bass_utils import run_bass_kernel_spmd

    frames = np.asarray(frames, dtype=np.float32)
    core = np.asarray(core, dtype=np.float32)

    if "nc" not in _compiled:
        _compiled["nc"] = _build_nc()
    nc = _compiled["nc"]

    in_maps = _host_prep(frames, core)
    trace = os.environ.get("KC_TRACE") == "1"
    tmpdir = os.environ.get("KC_TRACE_DIR") or None
    if tmpdir:
        os.makedirs(tmpdir, exist_ok=True)
    res = run_bass_kernel_spmd(nc, in_maps, list(range(NCORES)), trace=trace,
                               tmpdir=tmpdir)
    last_results = res

    G = NCORES * IMGS_PER_CORE
    out = np.empty((G, H, W), np.float32)
    for c in range(NCORES):
        o = res.results[c]["oout"]  # [2, 128, 2048] f16; rows are 4p+r in order
        for img in range(IMGS_PER_CORE):
            out[c * IMGS_PER_CORE + img] = o[img].reshape(H, W).astype(np.float32)
    return out.reshape(4, 4, H, W)


# revision 7
# speedup vs baseline: 1.2131x; 1.1395x over previous
"""Per-pixel adaptive 5x5 conv (KPN) for Trainium2, 8-core data parallel.

out[g,h,w] = sum_{i,j} core[g,5i+j,h,w] * frames_pad[g,h+i-2,w+j-2]
with g = flattened (B,N) = 16 image planes; 2 planes per NeuronCore.

Engine split (v6):
  DVE    : the 25 per-tap multiplies per image (fp16 taps in 2x_1P mode,
           the 5 fp8-weight taps at 1x), plus the last image's PSUM casts.
  TensorE: all tap accumulation, via identity-matmul into PSUM (fp32
           accumulate; 4 matmuls of FD=512 per tap = 1 PSUM bank each).
  ScalarE: builds the odd-parity frame copy on-chip (saves 2.1 MB of HBM
           per core), img0's PSUM casts, output DMA ring.
  GpSimd : idle (shares SBUF ports with DVE; using it slows DVE).

The end-to-end time is paced by the HBM weight stream through the 16
SDMA engines -- one of which (serving partitions 120-127) runs ~17%
slower and gates every transfer-completion semaphore. So the kernel
minimizes streamed bytes: tap group 0 of each image ships as fp8-e4m3
(halves those bytes; whole-output rel err ~1.2e-2, within the 2e-2
budget), the odd-parity frame copy is built on-chip, outputs are fp16.
The last weight group streams per-tap so the post-stream tail is one
tap, not five. Taps go even-j first so the parity copy has slack.

Layout: rows interleaved 4-per-partition. Partition p holds padded rows
4p..4p+7 (= orig rows 4p-2..4p+5), so ALL row shifts i=0..4 are free-dim
offsets -- no cross-partition moves and no per-shift duplication.
  fin  [2, 128, 8*518] fp16: fin[img,p,row*518+col] = Fpad[img,4p+row,col+1]
  win8 [2, 128, 5*4*512] fp8e4: tap group 0,  win8[img,p,k,r,c]
  win  [2, 4, 128, 5*4*512] fp16: tap groups 1-4
  oout [2, 128, 4*512] fp16: oout[img,p,r*512+c] = out[img,4p+r,c]
"""

import os
import sys

import numpy as np

for _p in ("/opt/trn_rl_repo",):
    if _p not in sys.path and os.path.isdir(_p):
        sys.path.insert(0, _p)

K = 5
NCORES = 8
IMGS_PER_CORE = 2
H = W = 512
RPP = 4          # output rows per partition
FROWS = RPP + K - 1  # 8 padded rows held per partition
FCOLS = 518
FH_FREE = FROWS * FCOLS  # 4144 elems per parity copy
W_FREE = K * RPP * W  # 10240
T_FREE = RPP * W  # 2048 (one tap's weights / one product / output)
KORDER = [0, 2, 4, 1, 3]  # even-j taps first within each group

_compiled = {}
last_results = None  # BassKernelResults of the most recent run (for test.py)


def _build_nc():
    import concourse.bacc as bacc
    import concourse.mybir as mybir
    from concourse.tile import TileContext

    f16 = mybir.dt.float16
    f32 = mybir.dt.float32
    f8 = mybir.dt.float8e4

    nc = bacc.Bacc(None, target_bir_lowering=False, debug=False)
    fin = nc.dram_tensor("fin", [IMGS_PER_CORE, 128, FH_FREE], f16,
                         kind="ExternalInput")
    win8 = nc.dram_tensor("win8", [IMGS_PER_CORE, 128, W_FREE], f8,
                          kind="ExternalInput")
    win = nc.dram_tensor("win", [IMGS_PER_CORE, K - 1, 128, W_FREE], f16,
                         kind="ExternalInput")
    iden = nc.dram_tensor("iden", [128, 128], f16, kind="ExternalInput")
    oout = nc.dram_tensor("oout", [IMGS_PER_CORE, 128, T_FREE], f16,
                          kind="ExternalOutput")

    with TileContext(nc) as tc:
        with (
            tc.tile_pool(name="idp", bufs=1) as idp,
            tc.tile_pool(name="fpool", bufs=1) as fpool,
            tc.tile_pool(name="w8p", bufs=2) as w8p,
            tc.tile_pool(name="wgrp", bufs=4) as wgrp,
            tc.tile_pool(name="wtap", bufs=5) as wtap,
            tc.tile_pool(name="prpool", bufs=6) as prpool,
            tc.tile_pool(name="opool", bufs=1) as opool,
            tc.psum_pool(name="ppool", bufs=1) as ppool,
        ):
            id_t = idp.tile([128, 128], f16)
            nc.scalar.dma_start(out=id_t[:], in_=iden[:])

            for img in range(IMGS_PER_CORE):
                split_tg = {K - 1} if img == IMGS_PER_CORE - 1 else set()

                f0_t = fpool.tile([128, FH_FREE], f16, tag=f"f0_{img}")
                nc.sync.dma_start(out=f0_t[:], in_=fin[img])
                # on-chip odd-parity copy: par1[c] = par0[c-1]; col 0 unread
                f1_t = fpool.tile([128, FH_FREE], f16, tag=f"f1_{img}")
                nc.scalar.copy(out=f1_t[:, 1:FH_FREE],
                               in_=f0_t[:, 0:FH_FREE - 1])
                fviews = [
                    f0_t[:].rearrange("p (row col) -> p row col", col=FCOLS),
                    f1_t[:].rearrange("p (row col) -> p row col", col=FCOLS),
                ]

                ps = ppool.tile([128, T_FREE], f32, tag=f"ps{img}")

                n_emitted = 0
                for tg in range(K):
                    if tg == 0:
                        w_t = w8p.tile([128, W_FREE], f8, tag="w8")
                        nc.sync.dma_start(out=w_t[:], in_=win8[img])
                        wv = w_t[:].rearrange("p (k r c) -> p k r c",
                                              k=K, r=RPP, c=W)
                        wviews = {k: wv[:, k] for k in KORDER}
                    elif tg in split_tg:
                        w_ts = {}
                        for k in KORDER:
                            w_t = wtap.tile([128, T_FREE], f16, tag="wt")
                            nc.sync.dma_start(
                                out=w_t[:],
                                in_=win[img, tg - 1][:, k * T_FREE:(k + 1) * T_FREE])
                            w_ts[k] = w_t
                        wviews = {k: w_ts[k][:].rearrange(
                            "p (r c) -> p r c", r=RPP) for k in KORDER}
                    else:
                        w_t = wgrp.tile([128, W_FREE], f16, tag="wg")
                        nc.sync.dma_start(out=w_t[:], in_=win[img, tg - 1])
                        wv = w_t[:].rearrange("p (k r c) -> p k r c",
                                              k=K, r=RPP, c=W)
                        wviews = {k: wv[:, k] for k in KORDER}

                    for k in KORDER:
                        i, j = tg, k
                        par = j & 1
                        joff = j + par
                        prod = prpool.tile([128, T_FREE], f16, tag="pr")
                        pv = prod[:].rearrange("p (r c) -> p r c", r=RPP)
                        f_ap = fviews[par][:, i:i + RPP, joff:joff + W]
                        nc.vector.tensor_mul(out=pv, in0=wviews[k], in1=f_ap)
                        for b in range(RPP):
                            nc.tensor.matmul(
                                ps[:, b * W:(b + 1) * W],
                                id_t[:],
                                prod[:, b * W:(b + 1) * W],
                                start=(n_emitted == 0),
                                stop=(n_emitted == K * K - 1),
                            )
                        n_emitted += 1

                o_t = opool.tile([128, T_FREE], f16, tag=f"o{img}")
                last_img = img == IMGS_PER_CORE - 1
                for b in range(RPP):
                    ob = o_t[:, b * W:(b + 1) * W]
                    pb = ps[:, b * W:(b + 1) * W]
                    if last_img:
                        nc.vector.tensor_copy(out=ob, in_=pb)
                    else:
                        nc.scalar.copy(out=ob, in_=pb)
                nc.scalar.dma_start(out=oout[img], in_=o_t[:])
    nc.finalize()
    return nc


def _host_prep(frames, core):
    """Build per-core in_maps. frames [4,4,1,512,512] f32, core [4,4,25,1,512,512]."""
    import concourse.mybir as mybir

    G = NCORES * IMGS_PER_CORE  # 16
    F = np.ascontiguousarray(frames.reshape(G, H, W))
    Wc = core.reshape(G, K * K, H, W)

    # frames: Fpad[g, R, C] = F[g, R-2, C-3]; rows pad 2/2, cols 3/4
    Fp = np.pad(F, ((0, 0), (2, 2), (3, 4))).astype(np.float16)  # [G,516,519]
    # 8-row windows starting at every 4th row: sw[g, p, row, col] = Fp[g, 4p+row, col]
    sw = np.lib.stride_tricks.sliding_window_view(Fp, FROWS, axis=1)
    sw = sw[:, ::RPP].transpose(0, 1, 3, 2)  # [G, 128, 8, 519]
    fprep = np.ascontiguousarray(sw[..., 1:1 + FCOLS])  # par=0: Fpad col c+1

    # weights: [g, tg, p, k, r, c] = core[g, 5tg+k, 4p+r, c]
    wall = Wc.reshape(G, K, K, 128, RPP, W).transpose(0, 1, 3, 2, 4, 5)
    f8np = mybir.dt.np(mybir.dt.float8e4)
    w8 = wall[:, 0].astype(f8np)          # [G, 128, 5, 4, 512] fp8
    w16 = wall[:, 1:].astype(np.float16)  # [G, 4, 128, 5, 4, 512] fp16

    iden = np.eye(128, dtype=np.float16)
    in_maps = []
    for c in range(NCORES):
        g0 = c * IMGS_PER_CORE
        in_maps.append({
            "fin": np.ascontiguousarray(
                fprep[g0:g0 + IMGS_PER_CORE].reshape(IMGS_PER_CORE, 128, FH_FREE)),
            "win8": np.ascontiguousarray(
                w8[g0:g0 + IMGS_PER_CORE].reshape(IMGS_PER_CORE, 128, W_FREE)),
            "win": np.ascontiguousarray(
                w16[g0:g0 + IMGS_PER_CORE].reshape(IMGS_PER_CORE, K - 1, 128, W_FREE)),
            "iden": iden,
        })
    return in_maps


def kernel(frames, core, bias):
    global last_results
    from concourse.bass_utils import run_bass_kernel_spmd

    frames = np.asarray(frames, dtype=np.float32)
    core = np.asarray(core, dtype=np.float32)

    if "nc" not in _compiled:
        _compiled["nc"] = _build_nc()
    nc = _compiled["nc"]

    in_maps = _host_prep(frames, core)
    trace = os.environ.get("KC_TRACE") == "1"
    tmpdir = os.environ.get("KC_TRACE_DIR") or None
    if tmpdir:
        os.makedirs(tmpdir, exist_ok=True)
    res = run_bass_kernel_spmd(nc, in_maps, list(range(NCORES)), trace=trace,
                               tmpdir=tmpdir)
    last_results = res

    G = NCORES * IMGS_PER_CORE
    out = np.empty((G, H, W), np.float32)
    for c in range(NCORES):
        o = res.results[c]["oout"]  # [2, 128, 2048] f16; rows are 4p+r in order
        for img in range(IMGS_PER_CORE):
            out[c * IMGS_PER_CORE + img] = o[img].reshape(H, W).astype(np.float32)
    return out.reshape(4, 4, H, W)


# revision 11
# speedup vs baseline: 1.2900x; 1.0634x over previous
"""Per-pixel adaptive 5x5 conv (KPN) for Trainium2, 8-core data parallel.

out[g,h,w] = sum_{i,j} core[g,5i+j,h,w] * frames_pad[g,h+i-2,w+j-2]
with g = flattened (B,N) = 16 image planes; 2 planes per NeuronCore.

Engine split (v6):
  DVE    : the 25 per-tap multiplies per image (fp16 taps in 2x_1P mode,
           the 5 fp8-weight taps at 1x), plus the last image's PSUM casts.
  TensorE: all tap accumulation, via identity-matmul into PSUM (fp32
           accumulate; 4 matmuls of FD=512 per tap = 1 PSUM bank each).
  ScalarE: builds the odd-parity frame copy on-chip (saves 2.1 MB of HBM
           per core), img0's PSUM casts, output DMA ring.
  GpSimd : idle (shares SBUF ports with DVE; using it slows DVE).

The end-to-end time is paced by the HBM weight stream through the 16
SDMA engines -- one of which (serving partitions 120-127) runs ~17%
slower and gates every transfer-completion semaphore. So the kernel
minimizes streamed bytes: tap group 0 of each image ships as fp8-e4m3
(halves those bytes; whole-output rel err ~1.2e-2, within the 2e-2
budget), the odd-parity frame copy is built on-chip, outputs are fp16.
The last weight group streams per-tap so the post-stream tail is one
tap, not five. Taps go even-j first so the parity copy has slack.

Layout: rows interleaved 4-per-partition. Partition p holds padded rows
4p..4p+7 (= orig rows 4p-2..4p+5), so ALL row shifts i=0..4 are free-dim
offsets -- no cross-partition moves and no per-shift duplication.
  fin  [2, 128, 8*518] fp16: fin[img,p,row*518+col] = Fpad[img,4p+row,col+1]
  win8 [2, 128, 5*4*512] fp8e4: tap group 0,  win8[img,p,k,r,c]
  win  [2, 4, 128, 5*4*512] fp16: tap groups 1-4
  oout [2, 128, 4*512] fp16: oout[img,p,r*512+c] = out[img,4p+r,c]
"""

import os
import sys

import numpy as np

for _p in ("/opt/trn_rl_repo",):
    if _p not in sys.path and os.path.isdir(_p):
        sys.path.insert(0, _p)

K = 5
NCORES = 8
IMGS_PER_CORE = 2
H = W = 512
RPP = 4          # output rows per partition
FROWS = RPP + K - 1  # 8 padded rows held per partition
FCOLS = 518
FH_FREE = FROWS * FCOLS  # 4144 elems per parity copy
W_FREE = K * RPP * W  # 10240
T_FREE = RPP * W  # 2048 (one tap's weights / one product / output)
KORDER = [0, 2, 4, 1, 3]  # even-j taps first within each group

_compiled = {}
last_results = None  # BassKernelResults of the most recent run (for test.py)


def _build_nc():
    import concourse.bacc as bacc
    import concourse.mybir as mybir
    from concourse.tile import TileContext

    f16 = mybir.dt.float16
    f32 = mybir.dt.float32
    f8 = mybir.dt.float8e4

    nc = bacc.Bacc(None, target_bir_lowering=False, debug=False)
    fin = nc.dram_tensor("fin", [IMGS_PER_CORE, 128, FH_FREE], f16,
                         kind="ExternalInput")
    win8 = nc.dram_tensor("win8", [IMGS_PER_CORE, 128, W_FREE], f8,
                          kind="ExternalInput")
    win = nc.dram_tensor("win", [IMGS_PER_CORE, K - 1, 128, W_FREE], f16,
                         kind="ExternalInput")
    iden = nc.dram_tensor("iden", [128, 128], f16, kind="ExternalInput")
    oout = nc.dram_tensor("oout", [IMGS_PER_CORE, 128, T_FREE], f16,
                          kind="ExternalOutput")

    with TileContext(nc) as tc:
        with (
            tc.tile_pool(name="idp", bufs=1) as idp,
            tc.tile_pool(name="fpool", bufs=1) as fpool,
            tc.tile_pool(name="w8p", bufs=1) as w8p,
            tc.tile_pool(name="w8tap", bufs=5) as w8tap,
            tc.tile_pool(name="wgrp", bufs=4) as wgrp,
            tc.tile_pool(name="wtap", bufs=5) as wtap,
            tc.tile_pool(name="prpool", bufs=6) as prpool,
            tc.tile_pool(name="opool", bufs=1) as opool,
            tc.psum_pool(name="ppool", bufs=1) as ppool,
        ):
            id_t = idp.tile([128, 128], f16)
            nc.scalar.dma_start(out=id_t[:], in_=iden[:])

            for img in range(IMGS_PER_CORE):
                split_tg = {K - 1} if img == IMGS_PER_CORE - 1 else set()

                f0_t = fpool.tile([128, FH_FREE], f16, tag=f"f0_{img}")
                nc.sync.dma_start(out=f0_t[:], in_=fin[img])
                # on-chip odd-parity copy: par1[c] = par0[c-1]; col 0 unread
                f1_t = fpool.tile([128, FH_FREE], f16, tag=f"f1_{img}")
                nc.scalar.copy(out=f1_t[:, 1:FH_FREE],
                               in_=f0_t[:, 0:FH_FREE - 1])
                fviews = [
                    f0_t[:].rearrange("p (row col) -> p row col", col=FCOLS),
                    f1_t[:].rearrange("p (row col) -> p row col", col=FCOLS),
                ]

                last_img = img == IMGS_PER_CORE - 1
                if last_img:
                    # per-bank PSUM tiles so the final casts/stores can
                    # pipeline with the last tap's matmuls
                    psb = [ppool.tile([128, W], f32, tag=f"psb{b}",
                                      name=f"psb{b}")
                           for b in range(RPP)]
                    ps_out = lambda b: psb[b][:]
                else:
                    ps = ppool.tile([128, T_FREE], f32, tag=f"ps{img}")
                    ps_out = lambda b: ps[:, b * W:(b + 1) * W]

                n_emitted = 0
                for tg in range(K):
                    if tg == 0:
                        if img == 0:
                            # per-tap fp8 chunks: compute starts ~3.5us sooner
                            w_ts = {}
                            for k in KORDER:
                                w_t = w8tap.tile([128, T_FREE], f8, tag="w8t")
                                nc.sync.dma_start(
                                    out=w_t[:],
                                    in_=win8[img][:, k * T_FREE:(k + 1) * T_FREE])
                                w_ts[k] = w_t
                            wviews = {k: w_ts[k][:].rearrange(
                                "p (r c) -> p r c", r=RPP) for k in KORDER}
                        else:
                            w_t = w8p.tile([128, W_FREE], f8, tag="w8")
                            nc.sync.dma_start(out=w_t[:], in_=win8[img])
                            wv = w_t[:].rearrange("p (k r c) -> p k r c",
                                                  k=K, r=RPP, c=W)
                            wviews = {k: wv[:, k] for k in KORDER}
                    elif tg in split_tg:
                        w_ts = {}
                        for k in KORDER:
                            w_t = wtap.tile([128, T_FREE], f16, tag="wt")
                            nc.sync.dma_start(
                                out=w_t[:],
                                in_=win[img, tg - 1][:, k * T_FREE:(k + 1) * T_FREE])
                            w_ts[k] = w_t
                        wviews = {k: w_ts[k][:].rearrange(
                            "p (r c) -> p r c", r=RPP) for k in KORDER}
                    else:
                        w_t = wgrp.tile([128, W_FREE], f16, tag="wg")
                        nc.sync.dma_start(out=w_t[:], in_=win[img, tg - 1])
                        wv = w_t[:].rearrange("p (k r c) -> p k r c",
                                              k=K, r=RPP, c=W)
                        wviews = {k: wv[:, k] for k in KORDER}

                    for k in KORDER:
                        i, j = tg, k
                        par = j & 1
                        joff = j + par
                        prod = prpool.tile([128, T_FREE], f16, tag="pr")
                        pv = prod[:].rearrange("p (r c) -> p r c", r=RPP)
                        f_ap = fviews[par][:, i:i + RPP, joff:joff + W]
                        nc.vector.tensor_mul(out=pv, in0=wviews[k], in1=f_ap)
                        for b in range(RPP):
                            nc.tensor.matmul(
                                ps_out(b),
                                id_t[:],
                                prod[:, b * W:(b + 1) * W],
                                start=(n_emitted == 0),
                                stop=(n_emitted == K * K - 1),
                            )
                        n_emitted += 1

                o_t = opool.tile([128, T_FREE], f16, tag=f"o{img}")
                if last_img:
                    # pipeline: cast banks on alternating engines as each
                    # bank's accumulation completes; store in two halves
                    # on separate DGE rings
                    for b in range(RPP):
                        ob = o_t[:, b * W:(b + 1) * W]
                        if b % 2 == 0:
                            nc.vector.tensor_copy(out=ob, in_=ps_out(b))
                        else:
                            nc.scalar.copy(out=ob, in_=ps_out(b))
                        if b == 1:
                            nc.sync.dma_start(out=oout[img][:, :2 * W],
                                              in_=o_t[:, :2 * W])
                    nc.scalar.dma_start(out=oout[img][:, 2 * W:],
                                        in_=o_t[:, 2 * W:])
                else:
                    for b in range(RPP):
                        nc.scalar.copy(out=o_t[:, b * W:(b + 1) * W],
                                       in_=ps_out(b))
                    nc.scalar.dma_start(out=oout[img], in_=o_t[:])
    nc.finalize()
    return nc


def _host_prep(frames, core):
    """Build per-core in_maps. frames [4,4,1,512,512] f32, core [4,4,25,1,512,512]."""
    import concourse.mybir as mybir

    G = NCORES * IMGS_PER_CORE  # 16
    F = np.ascontiguousarray(frames.reshape(G, H, W))
    Wc = core.reshape(G, K * K, H, W)

    # frames: Fpad[g, R, C] = F[g, R-2, C-3]; rows pad 2/2, cols 3/4
    Fp = np.pad(F, ((0, 0), (2, 2), (3, 4))).astype(np.float16)  # [G,516,519]
    # 8-row windows starting at every 4th row: sw[g, p, row, col] = Fp[g, 4p+row, col]
    sw = np.lib.stride_tricks.sliding_window_view(Fp, FROWS, axis=1)
    sw = sw[:, ::RPP].transpose(0, 1, 3, 2)  # [G, 128, 8, 519]
    fprep = np.ascontiguousarray(sw[..., 1:1 + FCOLS])  # par=0: Fpad col c+1

    # weights: [g, tg, p, k, r, c] = core[g, 5tg+k, 4p+r, c]
    wall = Wc.reshape(G, K, K, 128, RPP, W).transpose(0, 1, 3, 2, 4, 5)
    f8np = mybir.dt.np(mybir.dt.float8e4)
    w8 = wall[:, 0].astype(f8np)          # [G, 128, 5, 4, 512] fp8
    w16 = wall[:, 1:].astype(np.float16)  # [G, 4, 128, 5, 4, 512] fp16

    iden = np.eye(128, dtype=np.float16)
    in_maps = []
    for c in range(NCORES):
        g0 = c * IMGS_PER_CORE
        in_maps.append({
            "fin": np.ascontiguousarray(
                fprep[g0:g0 + IMGS_PER_CORE].reshape(IMGS_PER_CORE, 128, FH_FREE)),
            "win8": np.ascontiguousarray(
                w8[g0:g0 + IMGS_PER_CORE].reshape(IMGS_PER_CORE, 128, W_FREE)),
            "win": np.ascontiguousarray(
                w16[g0:g0 + IMGS_PER_CORE].reshape(IMGS_PER_CORE, K - 1, 128, W_FREE)),
            "iden": iden,
        })
    return in_maps


def kernel(frames, core, bias):
    global last_results
    from concourse.bass_utils import run_bass_kernel_spmd

    frames = np.asarray(frames, dtype=np.float32)
    core = np.asarray(core, dtype=np.float32)

    if "nc" not in _compiled:
        _compiled["nc"] = _build_nc()
    nc = _compiled["nc"]

    in_maps = _host_prep(frames, core)
    trace = os.environ.get("KC_TRACE") == "1"
    tmpdir = os.environ.get("KC_TRACE_DIR") or None
    if tmpdir:
        os.makedirs(tmpdir, exist_ok=True)
    res = run_bass_kernel_spmd(nc, in_maps, list(range(NCORES)), trace=trace,
                               tmpdir=tmpdir)
    last_results = res

    G = NCORES * IMGS_PER_CORE
    out = np.empty((G, H, W), np.float32)
    for c in range(NCORES):
        o = res.results[c]["oout"]  # [2, 128, 2048] f16; rows are 4p+r in order
        for img in range(IMGS_PER_CORE):
            out[c * IMGS_PER_CORE + img] = o[img].reshape(H, W).astype(np.float32)
    return out.reshape(4, 4, H, W)
